# revision 94
# baseline (speedup 1.0000x reference)
"""Multi-head attention (B=2, S=2048, D=1024, H=16, Dh=64) on 8 TRN2 cores.

Sharding: data-parallel over batch (2) x tensor-parallel over heads (16 -> 4
groups of 4). Core c handles batch c//4, heads [4*(c%4), 4*(c%4)+4).
Each core computes its partial output projection (Wo column slice); the host
sums the 4 partials per batch (the "all-reduce") and adds bo.

Device-side per core (fp16 streams, f32 PSUM/scores/denominators):
  Q_T/K_T/V_T = W @ X.T via PE, V transposed back to [s, j] via PE transpose.
  Per head: scores_T[k,q] = (K_T-tile).T @ Q_T (K=64 contraction, psum f32),
  exp on ACT (no max subtraction: scores ~ N(0,1), exp never overflows f32),
  attn@V with a memset ones-column appended to V so row 64 of the PSUM
  accumulator collects the softmax denominator; normalize via DVE
  reciprocal + a Pool-engine partition_broadcast + DVE multiply; the Wo
  projection and the output are fp16 (rel err ~7e-4, well under the gate).

The schedule is balanced against three serialized resources the cost model
exposes: the DMA bus (~360GB/s, ALL transfers serialize on one timeline),
the ACT engine (exp is 1038ns per [128,1024] tile = the attention pacer),
and PE (165us busy).
  - one explicit just-in-time DMA program on the SP queue: weights spliced
    into the x-chunk stream at the points where they hide, wo after the
    whole x stream; the pre-attention phase is DMA-bound (~14MB must land),
    so nothing else may touch the bus before the xq tail
  - V, K projected in full; Q projected only for jb0 (heads 0/1) using the
    attention pools' PSUM bufs (the big projection pool releases during Q,
    so the attention pipeline never waits on a PSUM pool turnover); Q-jb1
    is recomputed from resident xq chunks as PE filler inside tasks 0-3
  - projection psum evacuations are emitted inside the final dc pass and
    alternate DVE/ACT so the next projection's bufs free one by one
  - attention tasks (2 q-halves x 4 heads) software-pipeline their
    boundaries two exp-tiles deep, and all PE filler (deferred Q groups,
    qh0's output projection) is split into <=2-matmul units so the 2-deep
    score buffer never lets ACT starve
  - normalization of task i runs during task i+1 (reciprocal on DVE,
    broadcast on the otherwise-idle Pool engine); the last task normalizes
    in 512-col halves pipelined into the tail output projection, which
    ships each 128KB half on parallel DGE queues as soon as it lands
"""

import numpy as np
from contextlib import ExitStack

import concourse.bass as bass
from concourse import bacc
import concourse.mybir as mybir
import concourse.tile as tile

F32 = mybir.dt.float32
F32R = mybir.dt.float32r
F16 = mybir.dt.float16
AF = mybir.ActivationFunctionType

ATT_F16 = True

B = 2
S = 2048
D = 1024
H = 16
DH = 64
NCORES = 8
HL = 4          # heads per core
J = HL * DH     # 256 local projection width
P = 128
KD = D // P     # 8 d-chunks
NS = S // 512   # 4 s-tiles of 512
KB = S // P     # 16 k-blocks
QH = S // 1024  # 2 q-halves of 1024
EB = D // P     # 8 e-blocks

XDT = F16 if ATT_F16 else F32R        # streamed x / w dtype for q,k,v path
EX_BUFS = 10 if ATT_F16 else 8


def build_nc():
    nc = bacc.Bacc()

    xq = nc.dram_tensor("xq", [P, KD, S], XDT, kind="ExternalInput")
    xk = nc.dram_tensor("xk", [P, KD, S], XDT, kind="ExternalInput")
    xv = nc.dram_tensor("xv", [P, KD, S], XDT, kind="ExternalInput")
    wq = nc.dram_tensor("wq", [P, KD, J], XDT, kind="ExternalInput")
    wk = nc.dram_tensor("wk", [P, KD, J], XDT, kind="ExternalInput")
    wv = nc.dram_tensor("wv", [P, KD, J], XDT, kind="ExternalInput")
    wo = nc.dram_tensor("wo", [P, 2, D], XDT, kind="ExternalInput")
    out_t = nc.dram_tensor("out_t", [EB, P, S], XDT, kind="ExternalOutput")

    with tile.TileContext(nc) as tc, ExitStack() as st:
        const = st.enter_context(tc.tile_pool(name="const", bufs=1))
        persist = st.enter_context(tc.tile_pool(name="persist", bufs=1))
        xpool = st.enter_context(tc.tile_pool(name="xstream", bufs=8 if ATT_F16 else 4))
        # xq chunks live until the deferred Q-jb1 groups consume them
        xqpool = st.enter_context(tc.tile_pool(name="xqstream", bufs=KD))

        wq_sb = const.tile([P, KD, J], XDT, tag="wq")
        wk_sb = const.tile([P, KD, J], XDT, tag="wk")
        wv_sb = const.tile([P, KD, J], XDT, tag="wv")
        wo_sb = const.tile([P, 2, D], XDT, tag="wo")

        # Q_T as 8 region tiles [P, 512] (jb x stl) so the deferred jb1
        # stl-groups (computed inside attention tasks 0-3) never create
        # false deps against earlier tasks' score matmuls
        qt_t = [[persist.tile([P, 512], XDT, tag=f"qt{jb}_{stl}",
                              name=f"qt{jb}_{stl}")
                 for stl in range(NS)] for jb in range(2)]
        kt_sb = persist.tile([P, 2, S], XDT, tag="kt")   # K_T
        vt_sb = persist.tile([P, 2, S], XDT, tag="vt")   # V_T, pre-transpose
        v_sb = persist.tile([P, KB, HL * (DH + 1)], XDT, tag="v")  # V + ones
        ao_sb = persist.tile([P, 2, S], XDT, tag="ao")   # normalized attn out ^T

        identity = const.tile([P, P], XDT, tag="ident")

        from concourse.masks import make_identity
        if ATT_F16:
            make_identity(nc, identity[:])
        else:
            make_identity(nc, identity[:].bitcast(F32))

        # --- the DMA program. The cost model (and hw, roughly) serializes
        # all transfers on one bus timeline at ~360GB/s, and ~14MB must land
        # before the attention phase can start — the pre-attention phase is
        # DMA-bound. So: ONE explicit just-in-time order on the SP queue,
        # weights spliced between the x-chunks that hide them, wo after the
        # whole x stream (first needed ~20us later), and no ones-column DMA
        # at all (Pool memsets below). xv0 is split so the opening matmul
        # waits on 256KB.
        xv_chunks = [xpool.tile([P, S], XDT, tag="xc", name=f"xv{dc}")
                     for dc in range(KD)]
        xk_chunks = [xpool.tile([P, S], XDT, tag="xc", name=f"xk{dc}")
                     for dc in range(KD)]
        xq_chunks = [xqpool.tile([P, S], XDT, tag="xq", name=f"xqc{dc}")
                     for dc in range(KD)]
        nc.sync.dma_start(out=wv_sb[:, 0:1, :], in_=wv[:, 0:1, :])
        nc.sync.dma_start(out=xv_chunks[0][:, 0:1024], in_=xv[:, 0, 0:1024])
        nc.sync.dma_start(out=xv_chunks[0][:, 1024:], in_=xv[:, 0, 1024:])
        stream = [
            (wv_sb[:, 1:2, :], wv[:, 1:2, :]),
            (xv_chunks[1][:], xv[:, 1, :]),
            (wv_sb[:, 2:, :], wv[:, 2:, :]),
            (xv_chunks[2][:], xv[:, 2, :]),
            (xv_chunks[3][:], xv[:, 3, :]),
            (wk_sb[:, 0:4, :], wk[:, 0:4, :]),
            (xv_chunks[4][:], xv[:, 4, :]),
            (xv_chunks[5][:], xv[:, 5, :]),
            (wk_sb[:, 4:, :], wk[:, 4:, :]),
            (xv_chunks[6][:], xv[:, 6, :]),
            (xk_chunks[0][:], xk[:, 0, :]),
            (xv_chunks[7][:], xv[:, 7, :]),
            (xk_chunks[1][:], xk[:, 1, :]),
            (xk_chunks[2][:], xk[:, 2, :]),
            (wq_sb[:, 0:4, :], wq[:, 0:4, :]),
            (xk_chunks[3][:], xk[:, 3, :]),
            (xk_chunks[4][:], xk[:, 4, :]),
            (xq_chunks[0][:], xq[:, 0, :]),
            (xk_chunks[5][:], xk[:, 5, :]),
            (xk_chunks[6][:], xk[:, 6, :]),
            (xk_chunks[7][:], xk[:, 7, :]),
            (xq_chunks[1][:], xq[:, 1, :]),
            (xq_chunks[2][:], xq[:, 2, :]),
            (xq_chunks[3][:], xq[:, 3, :]),
            (wq_sb[:, 4:, :], wq[:, 4:, :]),
        ] + [(xq_chunks[dc][:], xq[:, dc, :]) for dc in range(4, KD)] + [
            (wo_sb[:], wo[:]),
        ]
        for dst_, src_ in stream:
            nc.sync.dma_start(out=dst_, in_=src_)

        def projection(chunks, wsb, dst, pproj, evac="dve",
                       pairs=None, after_dc=None):
            if pairs is None:
                pairs = [(jb, stl) for sg in range(NS // 2)
                         for jb in range(2)
                         for stl in (2 * sg, 2 * sg + 1)]
            psums = {
                (jb, stl): pproj.tile([P, 512], F32, tag="pp",
                                      name=f"pp{jb}_{stl}")
                for jb, stl in pairs
            }
            # each evacuation is emitted right after that psum's stop-matmul
            # in the final dc pass, alternating DVE/ACT, so the copies drain
            # while the remaining dc-7 matmuls still run and the next
            # projection's psum bufs free up one by one
            for dc in range(KD):
                for i, (jb, stl) in enumerate(pairs):
                    nc.tensor.matmul(
                        psums[jb, stl][:],
                        wsb[:, dc, jb * P:(jb + 1) * P],
                        chunks[dc][:, stl * 512:(stl + 1) * 512],
                        start=(dc == 0),
                        stop=(dc == KD - 1),
                    )
                    if dc == KD - 1:
                        d = dst(jb, stl)
                        if evac == "split" and i % 2 == 1:
                            nc.scalar.copy(d, psums[jb, stl][:])
                        else:
                            nc.vector.tensor_copy(d, psums[jb, stl][:])
                if after_dc is not None and dc in after_dc:
                    after_dc[dc]()

        # the softmax-denominator ones columns: Pool memsets, no DMA traffic
        for h in range(HL):
            nc.gpsimd.memset(v_sb[:, :, h * (DH + 1) + DH], 1.0)

        with tc.tile_pool(name="pproj", bufs=8, space="PSUM") as pproj:
            # order V, K, Q-jb0: K's evacs land during the Q projection, and
            # only Q's 4 evacs gate the attention start
            projection(
                xv_chunks, wv_sb,
                lambda jb, stl: vt_sb[:, jb, stl * 512:(stl + 1) * 512],
                pproj, evac="split",
            )
            projection(
                xk_chunks, wk_sb,
                lambda jb, stl: kt_sb[:, jb, stl * 512:(stl + 1) * 512],
                pproj, evac="split",
            )


        # --- attention + deferred V pipeline + interleaved output proj ---
        with tc.tile_pool(name="psc", bufs=2, space="PSUM") as psc, tc.tile_pool(
            name="poacc", bufs=2, space="PSUM"
        ) as poacc, tc.tile_pool(name="expp", bufs=EX_BUFS) as expp, tc.tile_pool(
            name="npool", bufs=4
        ) as npool, tc.tile_pool(name="ostage", bufs=8) as opool:

            # Q-jb0 pre-projection, using the attention pools' psum bufs:
            # the big pproj pool releases while this runs, so the attention
            # pipeline never stalls on a PSUM pool turnover, and the first
            # scores wait only on qt data (region deps), not pool gates
            qpre = {}
            for s in range(NS):
                pp = psc if s < 2 else poacc
                qpre[s] = pp.tile([P, 512], F32,
                                  tag="sc" if pp is psc else "oacc",
                                  name=f"qpre{s}")
            for dc in range(KD):
                for s in range(NS):
                    nc.tensor.matmul(
                        qpre[s][:],
                        wq_sb[:, dc, 0:P],
                        xq_chunks[dc][:, s * 512:(s + 1) * 512],
                        start=(dc == 0),
                        stop=(dc == KD - 1),
                    )
                    if dc == KD - 1:
                        if s % 2 == 1:
                            nc.scalar.copy(qt_t[0][s][:], qpre[s][:])
                        else:
                            nc.vector.tensor_copy(qt_t[0][s][:], qpre[s][:])

            # V transposes, emitted lazily inside the first heads' kb-loops
            # (vt_sb is ready before attention starts; these fill PE slack
            # and borrow the spare "oacc" PSUM slot)
            def vjob_transpose(sb, jb):
                def f():
                    tp = poacc.tile([P, P], XDT, tag="oacc",
                                    name=f"tp_{sb}_{jb}")
                    nc.tensor.transpose(
                        tp[:, :P], vt_sb[:, jb, sb * P:(sb + 1) * P], identity[:]
                    )
                    for hh in range(2):
                        h = jb * 2 + hh
                        nc.vector.tensor_copy(
                            v_sb[:, sb, h * (DH + 1):h * (DH + 1) + DH],
                            tp[:, hh * DH:(hh + 1) * DH],
                        )
                return f

            vjobs = []
            for sb in range(KB):
                vjobs.append(vjob_transpose(sb, 0))
                vjobs.append(vjob_transpose(sb, 1))

            def qjob_units(jb, stl):
                # deferred Q (jb, stl)-group, split into 2-matmul units so
                # each injection stalls the score pipeline by less than one
                # exp tile (the score buffer is only 2 deep). Accumulates in
                # the spare poacc slot; evac to qt_t[jb][stl] on DVE.
                box = []

                def mk(d0):
                    def f():
                        if not box:
                            box.append(poacc.tile([P, 512], F32, tag="oacc",
                                                  name=f"qg{jb}_{stl}"))
                        ps = box[0]
                        for dc in range(d0, d0 + 2):
                            nc.tensor.matmul(
                                ps[:],
                                wq_sb[:, dc, jb * P:(jb + 1) * P],
                                xq_chunks[dc][:, stl * 512:(stl + 1) * 512],
                                start=(dc == 0),
                                stop=(dc == KD - 1),
                            )
                        if d0 + 2 == KD:
                            nc.vector.tensor_copy(qt_t[jb][stl][:], ps[:])
                    return f

                return [mk(d0) for d0 in range(0, KD, 2)]

            def oproj_units(eb):
                # qh0's output projection for one eb, as two (stl) units
                box = []

                def mk(stl):
                    def f():
                        if not box:
                            box.append(opool.tile([P, 1024], XDT, tag="ob",
                                                  name=f"ob0_{eb}"))
                        ob = box[0]
                        po = poacc.tile([P, 512], F32, tag="oacc",
                                        name=f"po0_{eb}_{stl}")
                        for jb in range(2):
                            nc.tensor.matmul(
                                po[:, :512],
                                wo_sb[:, jb, eb * P:(eb + 1) * P],
                                ao_sb[:, jb, stl * 512:(stl + 1) * 512],
                                start=(jb == 0),
                                stop=(jb == 1),
                            )
                        nc.vector.tensor_copy(
                            ob[:, stl * 512:(stl + 1) * 512], po[:, :512])
                        if stl == 1:
                            nc.sync.dma_start(out=out_t[eb][:, 0:1024],
                                              in_=ob[:])
                    return f

                return [mk(0), mk(1)]

            def scores_exp(qh, h, kb, pool=None):
                jb = h // 2
                off = DH * (h % 2)
                q0 = qh * 1024
                pool = pool if pool is not None else psc
                sc = pool.tile([P, 1024], F32,
                               tag="sc" if pool is psc else "oacc",
                               name=f"sc{qh}_{h}_{kb}")
                for n in range(2):
                    nc.tensor.matmul(
                        sc[:, n * 512:(n + 1) * 512],
                        kt_sb[off:off + DH, jb, kb * P:(kb + 1) * P],
                        qt_t[jb][q0 // 512 + n][off:off + DH, :],
                        start=True,
                        stop=True,
                    )
                ex = expp.tile([P, 1024], XDT, tag="ex",
                               name=f"ex{qh}_{h}_{kb}")
                nc.scalar.activation(ex[:], sc[:], AF.Exp)
                return ex

            def kb_loop(qh, h, vjob_budget=0, fillers=None, norm_cb=None,
                        split_recip=False, pre_ex=(), next_task=None):
                oacc = poacc.tile([DH + 1, 1024], F32, tag="oacc")
                ex_next = []
                for kb in range(KB):
                    ex = pre_ex[kb] if kb < len(pre_ex) else \
                        scores_exp(qh, h, kb)
                    for _ in range(vjob_budget):
                        if vjobs:
                            vjobs.pop(0)()
                    if norm_cb is not None and kb == KB // 4:
                        norm_cb()
                    if fillers is not None:
                        for f in fillers.get(kb, []):
                            f()
                    if kb >= KB - 2 and next_task is not None:
                        # software-pipeline the task boundary 2 tiles deep
                        # (emitted after this task's fillers, so every qt
                        # region they read is already written): ACT never
                        # drains while PE works off its filler lag
                        ex_next.append(scores_exp(*next_task, kb - (KB - 2)))
                    for n in range(2):
                        nc.tensor.matmul(
                            oacc[:, n * 512:(n + 1) * 512],
                            v_sb[:, kb, h * (DH + 1):(h + 1) * (DH + 1)],
                            ex[:, n * 512:(n + 1) * 512],
                            start=(kb == 0),
                            stop=(kb == KB - 1),
                        )
                if split_recip:
                    # the last task's 1/denom in halves: the Pool broadcast
                    # for half 0 starts while DVE still computes half 1
                    recips = []
                    for hf in range(2):
                        r = npool.tile([1, 512], F32R, tag="recip",
                                       name=f"recip_h{hf}")
                        with nc.allow_low_precision(reason="f32r denom"):
                            nc.vector.reciprocal(
                                r[:], oacc[DH:DH + 1, hf * 512:(hf + 1) * 512]
                            )
                        recips.append(r)
                    return oacc, recips, ex_next
                recip = npool.tile([1, 1024], F32R, tag="recip")
                with nc.allow_low_precision(reason="fp32r softmax denom"):
                    nc.vector.reciprocal(recip[:], oacc[DH:DH + 1, :])
                return oacc, [recip], ex_next

            def normalize(task_state):
                # partition-broadcast the 1/denom row on the (idle) Pool
                # engine: no PE matmul, no PSUM tile, no DVE copy
                (qh, h), (oacc, recips) = task_state
                q0 = qh * 1024
                jb = h // 2
                off = DH * (h % 2)
                bcast = npool.tile([DH, 1024], F32R, tag="bcast")
                nc.gpsimd.partition_broadcast(bcast[:], recips[0][:])
                nc.vector.tensor_mul(
                    ao_sb[off:off + DH, jb, q0:q0 + 1024],
                    oacc[0:DH, :],
                    bcast[:],
                )

            def oproj_slice(qh, ebs, stls=(0, 1)):
                q0 = qh * 1024
                for eb in ebs:
                    ob = opool.tile([P, 1024], XDT, tag="ob",
                                    name=f"ob{qh}_{eb}_{stls[0]}")
                    for stl in stls:
                        s0 = q0 + stl * 512
                        # at the tail (qh=1) the score pool is retired, so
                        # alternate po tiles across both PSUM pools for a
                        # 4-deep matmul/evac pipeline
                        pp = psc if qh == 1 and eb % 2 == 0 else poacc
                        po = pp.tile([P, 512], F32,
                                     tag="sc" if pp is psc else "oacc",
                                     name=f"po_{qh}_{eb}_{stl}")
                        for jb in range(2):
                            nc.tensor.matmul(
                                po[:, :512],
                                wo_sb[:, jb, eb * P:(eb + 1) * P],
                                ao_sb[:, jb, s0:s0 + 512],
                                start=(jb == 0),
                                stop=(jb == 1),
                            )
                        d = ob[:, stl * 512:(stl + 1) * 512]
                        if qh == 1 and (eb + stl) % 2 == 0:
                            nc.scalar.copy(d, po[:, :512])  # ACT idle at tail
                        else:
                            nc.vector.tensor_copy(d, po[:, :512])
                        if qh == 1:
                            # per-stl DMA, alternating DGE queues so the
                            # tail generations run in parallel
                            eng = nc.gpsimd if eb % 2 else nc.sync
                            eng.dma_start(out=out_t[eb][:, s0:s0 + 512],
                                          in_=d)
                    if qh == 0:
                        nc.sync.dma_start(out=out_t[eb][:, q0:q0 + 1024],
                                          in_=ob[:])

            tasks = [(qh, h) for qh in range(QH) for h in range(HL)]
            pending = [None]
            ex_hand = ()
            for i, (qh, h) in enumerate(tasks):
                # sprinkle V transposes into the first task's PE slack;
                # the previous task's normalize lands at kb=4 (frees its
                # accumulator slot); deferred Q-jb1 group i fills task i
                # (0-3) and qh0's output projection fills tasks 4-7, both
                # as 2-matmul units spread across kbs
                def norm_prev():
                    if pending[0] is not None:
                        normalize(pending[0])
                        pending[0] = None
                qdefer = [(1, 0), (1, 1), (1, 2), (1, 3)]
                if i < 4:
                    units = qjob_units(*qdefer[i])
                    kbs = (8, 10, 12, 14)
                else:
                    units = (oproj_units((i - 4) * 2)
                             + oproj_units((i - 4) * 2 + 1))
                    kbs = (8, 10, 12, 14)
                fillers = {kb: [u] for kb, u in zip(kbs, units)}
                nxt = tasks[i + 1] if i + 1 < len(tasks) else None
                oacc_i, recips_i, ex_hand = kb_loop(
                    qh, h, vjob_budget=6 if i < 1 else 0,
                    fillers=fillers, norm_cb=norm_prev,
                    split_recip=(i == len(tasks) - 1),
                    pre_ex=ex_hand, next_task=nxt,
                )
                assert not vjobs or i < 1
                pending[0] = ((qh, h), (oacc_i, recips_i))
            # tail: the last task normalizes in 512-col halves; each half's
            # output projection runs while the next half's broadcast/mul is
            # still on Pool/DVE. Staging tiles are shared across the two
            # passes so each eb ships as ONE 256KB DMA (the per-DMA
            # generation cost dominates smaller transfers), alternating the
            # SP/ACT hardware DGE queues.
            (qh_l, h_l), (oacc_l, recips_l) = pending[0]
            q0_l = qh_l * 1024
            jb_l = h_l // 2
            off_l = DH * (h_l % 2)
            ob_tail = [opool.tile([P, 1024], XDT, tag="ob", name=f"obt{eb}")
                       for eb in range(EB)]
            for half in range(2):
                c0 = half * 512
                s0 = q0_l + c0
                bcast = npool.tile([DH, 512], F32R, tag="bcast",
                                   name=f"bct{half}")
                nc.gpsimd.partition_broadcast(bcast[:], recips_l[half][:])
                nc.vector.tensor_mul(
                    ao_sb[off_l:off_l + DH, jb_l, s0:s0 + 512],
                    oacc_l[0:DH, c0:c0 + 512],
                    bcast[:],
                )
                for eb in range(EB):
                    # score pool is retired at the tail: alternate po tiles
                    # across both PSUM pools for a deeper matmul/evac pipe
                    pp = psc if eb % 2 == 0 else poacc
                    po = pp.tile([P, 512], F32,
                                 tag="sc" if pp is psc else "oacc",
                                 name=f"pot_{eb}_{half}")
                    for jb in range(2):
                        nc.tensor.matmul(
                            po[:, :512],
                            wo_sb[:, jb, eb * P:(eb + 1) * P],
                            ao_sb[:, jb, s0:s0 + 512],
                            start=(jb == 0),
                            stop=(jb == 1),
                        )
                    d = ob_tail[eb][:, c0:c0 + 512]
                    if eb % 2 == 0:
                        nc.scalar.copy(d, po[:, :512])  # ACT idle at tail
                    else:
                        nc.vector.tensor_copy(d, po[:, :512])
                    # ship each 128KB half as soon as its evac lands — the
                    # tail is DMA-bandwidth-bound, so bytes must start
                    # moving during pass 0, on both hardware DGE queues
                    if half:
                        eng = nc.scalar if eb % 2 else nc.sync
                    else:
                        eng = nc.gpsimd
                    eng.dma_start(out=out_t[eb][:, s0:s0 + 512], in_=d)

    nc.finalize()
    return nc


_NC_CACHE = None


def _get_nc():
    global _NC_CACHE
    if _NC_CACHE is None:
        _NC_CACHE = build_nc()
    return _NC_CACHE


def make_in_maps(query, key, value, Wq, Wk, Wv, Wo):
    """Build the 8 per-core input dicts from the full tensors (p-major)."""
    query = np.asarray(query, np.float32)
    key = np.asarray(key, np.float32)
    value = np.asarray(value, np.float32)
    Wq = np.asarray(Wq, np.float32)
    Wk = np.asarray(Wk, np.float32)
    Wv = np.asarray(Wv, np.float32)
    Wo = np.asarray(Wo, np.float32)
    xdt = np.float16 if ATT_F16 else np.float32

    def pmajor(a2d, inner):  # [Drows, inner] -> [P, Drows//P, inner]
        return np.ascontiguousarray(
            a2d.reshape(KD, P, inner).transpose(1, 0, 2)
        )

    scale = np.float32(1.0 / np.sqrt(DH))
    xs = {}
    for b in range(B):
        xs[b] = {
            "xq": pmajor(np.ascontiguousarray(query[b].T), S).astype(xdt),
            "xk": pmajor(np.ascontiguousarray(key[b].T), S).astype(xdt),
            "xv": pmajor(np.ascontiguousarray(value[b].T), S).astype(xdt),
        }
    ws = {}
    for hg in range(4):
        sl = slice(hg * J, (hg + 1) * J)
        wo_t = np.ascontiguousarray(Wo[:, sl].T)  # [256, 1024]
        ws[hg] = {
            "wq": pmajor(np.ascontiguousarray(Wq[sl].T * scale), J).astype(xdt),
            "wk": pmajor(np.ascontiguousarray(Wk[sl].T), J).astype(xdt),
            "wv": pmajor(np.ascontiguousarray(Wv[sl].T), J).astype(xdt),
            "wo": np.ascontiguousarray(
                wo_t.reshape(2, P, D).transpose(1, 0, 2)
            ).astype(xdt),
        }
    in_maps = []
    for c in range(NCORES):
        b, hg = c // 4, c % 4
        m = {}
        m.update(xs[b])
        m.update(ws[hg])
        in_maps.append(m)
    return in_maps


def assemble(results, bo):
    """Sum the 4 per-core partials per batch, add bo."""
    bo = np.asarray(bo, np.float32)
    out = np.zeros((B, S, D), np.float32)
    for c in range(NCORES):
        b = c // 4
        part = results[c]["out_t"].astype(np.float32).reshape(D, S).T  # [S, D]
        out[b] += part
    out += bo[None, None, :]
    return out


def kernel(query, key, value, Wq, Wk, Wv, Wo, bo):
    import os
    import time

    # helps recover wedged NeuronCores between runs
    os.environ.setdefault("NEURON_RT_RESET_CORES", "1")
    from concourse.bass_utils import run_bass_kernel_spmd

    nc = _get_nc()
    in_maps = make_in_maps(query, key, value, Wq, Wk, Wv, Wo)
    last_exc = None
    for attempt in range(3):
        try:
            res = run_bass_kernel_spmd(nc, in_maps, list(range(NCORES)))
            return assemble(res.results, bo)
        except Exception as e:  # transient NRT_EXEC_UNIT_UNRECOVERABLE etc.
            last_exc = e
            time.sleep(2.0)
    raise last_exc

